# revision 5
# baseline (speedup 1.0000x reference)
"""AlignmentLoss on 8 Trainium2 cores.

Edit-distance DP over xent substitution costs, data-parallel over batch
(4 batch elements per core). Per DP row the recurrence
    D[i][j] = min(D[i-1][j-1] + sub, D[i-1][j] + 1, D[i][j-1] + ins[j-1])
is computed as two elementwise DVE ops + one native (min,+) scan
(tensor_tensor_scan).  The token gather sub[i][j] = -logp[b, j, tok[b,i]]
is produced by the tensor engine: a block-diagonal one-hot lhsT
([32 tok x 4 batch] = K=128) against logpT [128=32b+t, 1024=j], giving each
row's substitution costs directly in [4, 1024] PSUM layout.

Everything value-dependent (one-hots, seq-len masks) is DMA'd data, so the
Bass program is compiled once at import time.  _build(reps=N) emits the whole
computation N times back-to-back — used by test.py to measure per-execution
device time by differencing (the axon dispatch overhead ~15ms/call would
otherwise swamp the ~1ms kernel).
"""
import numpy as np

B, M, N, T = 32, 512, 1024, 32
NCORES = 8
BL = B // NCORES  # batches per core
PAD = 1
DEL = 1.0
EPS = 1e-7
INF = 1e9

_STATE = {}


def _build(reps=1):
    from contextlib import ExitStack
    import concourse.bacc as bacc
    import concourse.tile as tile
    from concourse import mybir

    F32 = mybir.dt.float32
    ADD = mybir.AluOpType.add
    SUB = mybir.AluOpType.subtract
    MIN = mybir.AluOpType.min
    MULT = mybir.AluOpType.mult
    MAX = mybir.AluOpType.max
    AX = mybir.AxisListType.X

    nc = bacc.Bacc("TRN2", target_bir_lowering=False, debug=False,
                   num_devices=NCORES)
    yp_d = nc.dram_tensor("yp", [BL * N, T], F32, kind="ExternalInput").ap()
    oh_d = nc.dram_tensor("oh", [128, BL * M], F32, kind="ExternalInput").ap()
    ohins_d = nc.dram_tensor("ohins", [128, BL], F32, kind="ExternalInput").ap()
    ident_d = nc.dram_tensor("ident", [128, 128], F32, kind="ExternalInput").ap()
    initcol_d = nc.dram_tensor("initcol", [BL, M + 1], F32, kind="ExternalInput").ap()
    maskcol_d = nc.dram_tensor("maskcol", [BL, M + 1], F32, kind="ExternalInput").ap()
    ans_d = nc.dram_tensor("ans", [BL, 1], F32, kind="ExternalOutput").ap()

    with tile.TileContext(nc) as tc:
        with ExitStack() as ctx:
            const = ctx.enter_context(tc.tile_pool(name="const", bufs=1))
            dpool = ctx.enter_context(tc.tile_pool(name="dpool", bufs=2))
            tpool = ctx.enter_context(tc.tile_pool(name="tpool", bufs=2))
            cpool = ctx.enter_context(tc.tile_pool(name="cpool", bufs=2))
            npool = ctx.enter_context(tc.tile_pool(name="npool", bufs=3))
            spool = ctx.enter_context(tc.tile_pool(name="spool", bufs=3))
            trp = ctx.enter_context(tc.tile_pool(name="trp", bufs=2, space="PSUM"))
            mpool = ctx.enter_context(tc.tile_pool(name="mpool", bufs=3, space="PSUM"))

            for _rep in range(reps):
                logpT = const.tile([128, N], F32, tag="logpT", name="logpT")
                ins_sb = const.tile([BL, N], F32, tag="ins_sb", name="ins_sb")
                initcol = const.tile([BL, M + 1], F32, tag="initcol", name="initcol")
                maskcol = const.tile([BL, M + 1], F32, tag="maskcol", name="maskcol")
                oh_sb = const.tile([128, BL * M], F32, tag="oh_sb", name="oh_sb")
                ohins = const.tile([128, BL], F32, tag="ohins", name="ohins")
                ident = const.tile([128, 128], F32, tag="ident", name="ident")
                colend = const.tile([BL, M + 1], F32, tag="colend", name="colend")
                infrow = const.tile([BL, N], F32, tag="infrow", name="infrow")

                nc.sync.dma_start(initcol[:], initcol_d[:])
                nc.sync.dma_start(maskcol[:], maskcol_d[:])
                nc.sync.dma_start(oh_sb[:], oh_d[:])
                nc.sync.dma_start(ohins[:], ohins_d[:])
                nc.sync.dma_start(ident[:], ident_d[:])
                nc.vector.memset(infrow[:], INF)

                # --- prologue: normalize, log, transpose into logpT[32b+t, j] ---
                for k in range(BL * N // 128):
                    b, c = k // (N // 128), k % (N // 128)
                    yt = npool.tile([128, T], F32, tag="yt", name="yt")
                    nc.sync.dma_start(yt[:], yp_d[128 * k:128 * (k + 1), :])
                    s = spool.tile([128, 1], F32, tag="s", name="s")
                    nc.vector.reduce_sum(s[:], yt[:], AX)
                    r = spool.tile([128, 1], F32, tag="r", name="r")
                    nc.vector.reciprocal(r[:], s[:])
                    yn = npool.tile([128, T], F32, tag="yn", name="yn")
                    nc.vector.tensor_scalar(yn[:], yt[:], r[:], None, MULT)
                    yc = npool.tile([128, T], F32, tag="yc", name="yc")
                    nc.vector.tensor_scalar(yc[:], yn[:], EPS, 1.0 - EPS, MAX, MIN)
                    lg = npool.tile([128, T], F32, tag="lg", name="lg")
                    nc.scalar.activation(lg[:], yc[:], mybir.ActivationFunctionType.Ln)
                    ptr = trp.tile([T, 128], F32, tag="ptr", name="ptr")
                    nc.tensor.transpose(ptr[:], lg[:], ident[:])
                    nc.vector.tensor_copy(
                        logpT[T * b:T * (b + 1), 128 * c:128 * (c + 1)], ptr[:])

                # insertion costs: ins[b, j] = -logp[b, j, PAD]
                ips = mpool.tile([BL, N], F32, tag="mps", name="ips")
                nc.tensor.matmul(ips[:, 0:512], ohins[:], logpT[:, 0:512],
                                 start=True, stop=True)
                nc.tensor.matmul(ips[:, 512:1024], ohins[:], logpT[:, 512:1024],
                                 start=True, stop=True)
                nc.vector.tensor_scalar(ins_sb[:], ips[:], -1.0, None, MULT)

                # --- DP ---
                # D0[j] = prefix sum of ins
                D = dpool.tile([BL, N + 1], F32, tag="D", name="D0")
                nc.vector.memset(D[:, 0:1], 0.0)
                nc.vector.tensor_tensor_scan(D[:, 1:N + 1], ins_sb[:], infrow[:],
                                             0.0, ADD, MIN)
                nc.scalar.copy(colend[:, 0:1], D[:, N:N + 1])

                for i in range(1, M + 1):
                    mps = mpool.tile([BL, N], F32, tag="mps", name="mps")
                    nc.tensor.matmul(mps[:, 0:512], oh_sb[:, BL * (i - 1):BL * i],
                                     logpT[:, 0:512], start=True, stop=True)
                    nc.tensor.matmul(mps[:, 512:1024], oh_sb[:, BL * (i - 1):BL * i],
                                     logpT[:, 512:1024], start=True, stop=True)
                    t = tpool.tile([BL, N], F32, tag="t", name="t")
                    nc.vector.tensor_tensor(t[:], D[:, 0:N], mps[:], SUB)
                    cnd = cpool.tile([BL, N], F32, tag="cnd", name="cnd")
                    nc.vector.scalar_tensor_tensor(cnd[:], D[:, 1:N + 1], 1.0, t[:],
                                                   ADD, MIN)
                    Dn = dpool.tile([BL, N + 1], F32, tag="D", name="D")
                    nc.vector.tensor_tensor_scan(Dn[:, 1:N + 1], ins_sb[:], cnd[:],
                                                 initcol[:, i:i + 1], ADD, MIN)
                    nc.scalar.copy(Dn[:, 0:1], initcol[:, i:i + 1])
                    nc.scalar.copy(colend[:, i:i + 1], Dn[:, N:N + 1])
                    D = Dn

                prod = const.tile([BL, M + 1], F32, tag="prod", name="prod")
                nc.vector.tensor_tensor(prod[:], colend[:], maskcol[:], MULT)
                ansT = const.tile([BL, 1], F32, tag="ansT", name="ansT")
                nc.vector.reduce_sum(ansT[:], prod[:], AX)
                nc.sync.dma_start(ans_d[:], ansT[:])

    nc.compile()
    return nc


def _get_state():
    if "nc" not in _STATE:
        _STATE["nc"] = _build()
    return _STATE


def _host_prep(y_true, y_pred):
    y_true = np.asarray(y_true)
    y_pred = np.asarray(y_pred, dtype=np.float32)
    ixs = np.arange(M)
    keys = np.where(y_true != PAD, ixs[None, :], M + ixs[None, :])
    order = np.sort(keys, axis=1) % M
    y_ls = np.take_along_axis(y_true, order, axis=1).astype(np.int64)
    seq_lens = np.sum(y_ls != PAD, axis=-1).astype(np.int64)

    ident = np.eye(128, dtype=np.float32)
    initcol = np.broadcast_to(np.arange(M + 1, dtype=np.float32), (BL, M + 1)).copy()
    ohins = np.zeros((128, BL), np.float32)
    for b in range(BL):
        ohins[32 * b + PAD, b] = 1.0

    in_maps = []
    for core in range(NCORES):
        sl = slice(core * BL, (core + 1) * BL)
        yls_c = y_ls[sl]
        L_c = seq_lens[sl]
        oh = np.zeros((128, BL * M), np.float32)
        cols = BL * np.arange(M)
        for b in range(BL):
            oh[32 * b + yls_c[b], cols + b] = 1.0
        maskcol = np.zeros((BL, M + 1), np.float32)
        maskcol[np.arange(BL), L_c] = 1.0
        in_maps.append({
            "yp": y_pred[sl].reshape(BL * N, T),
            "oh": oh,
            "ohins": ohins,
            "ident": ident,
            "initcol": initcol,
            "maskcol": maskcol,
        })
    return in_maps


def kernel(y_true, y_pred, _trace=False):
    from concourse import bass_utils
    st = _get_state()
    in_maps = _host_prep(y_true, y_pred)
    res = bass_utils.run_bass_kernel_spmd(
        st["nc"], in_maps, core_ids=list(range(NCORES)), trace=_trace)
    if _trace:
        _STATE["last_result"] = res
    total = np.float64(0.0)
    for core in range(NCORES):
        total += np.float64(res.results[core]["ans"]).sum()
    return np.float32(total)
